# revision 1
# baseline (speedup 1.0000x reference)
"""Cross-conditional GPT2 sparse attention block on 8 Trainium2 NeuronCores.

Sharding: core = (batch b in 0..3) x (head-group g in 0..1, 6 heads each).
Each core computes, for its (b, g):
  qT/kT = (Wq_g @ x_b^T + bq_g)  laid out [d_on_partitions, L]
  v     = x_b @ Wv_g^T + bv_g    natural layout [L, 384], interleaved with a
          ones column per head ([L, 6, 65]) so att@v also yields the softmax
          denominator for free.
  scores are computed *transposed* (sT[j, i]) so that softmax needs no
  transpose at all: exp on ACT, multiplicative 0/1 mask (host-built, bf16),
  att@v via lhsT=v (natural layout), denominator broadcast across partitions
  via a K=1 PE matmul, then the partial output projection with Wp[:, g]^T.
Host sums the two per-batch partials and adds bp.
"""

import sys

sys.path.insert(0, "/opt/trn_rl_repo")

from contextlib import ExitStack

import ml_dtypes
import numpy as np

import concourse.bacc as bacc
import concourse.bass as bass
import concourse.mybir as mybir
import concourse.tile as tile
from concourse.bass_utils import run_bass_kernel_spmd

# ---- problem constants (hardcoded per spec) ----
B = 4
T = 512
N = 8
C = 768
NHEAD = 12
L = 3 * T + 4 * N  # 1568
P = 128
G = C // 2  # 384 channels per head-group
NH = 6  # heads per core
D = 64  # head dim
ET = C // P  # 6 e-tiles (contraction of x @ W)
CT = G // P  # 3 c-tiles of the group's channels
NJT = (L + P - 1) // P  # 13 j tiles (12x128 + 32)
JPAD = NJT * P  # 1664
I_CHUNKS = [(0, 512), (512, 512), (1024, 512), (1536, 32)]
SCALE = 1.0 / 8.0  # 1/sqrt(64)

F32 = mybir.dt.float32
BF16 = mybir.dt.bfloat16
F16 = mybir.dt.float16

_NC = None  # cached compiled Bass program


def _jl(jt):
    return P if jt < NJT - 1 else L - (NJT - 1) * P  # 128 or 32


def _score_intervals(jt):
    """i-ranges (start, len) that can attend any column in j-tile jt.
    Derived from the cross-conditional mask block structure. The text-row
    strip [1536,1568) is merged into the preceding torso interval whenever
    the combined length fits one PSUM bank (<=512)."""
    if jt <= 3:
        j0 = jt * P
        iv = [(j0, 512 - j0), (512 + j0, 512 - j0), (1024 + j0, 512 - j0), (1536, 32)]
    elif jt <= 11:
        f0 = (jt % 4) * P
        iv = [(512 + f0, 512 - f0), (1024 + f0, 512 - f0), (1536, 32)]
    else:
        iv = [(512, 512), (1024, 512), (1536, 32)]
    if len(iv) >= 2 and iv[-2][0] + iv[-2][1] == 1536 and iv[-2][1] + 32 <= 512:
        iv = iv[:-2] + [(iv[-2][0], iv[-2][1] + 32)]
    return iv


def _ich_of(a):
    return 3 if a == 1536 else a // 512


_ATTV_LAST = {0: 3, 1: NJT - 1, 2: NJT - 1, 3: NJT - 1}  # last jt per ich

# (group) -> per-jt score interval (a, ln) and mask spec.
# g0 = upper rows (i 0..512), jts 0..3; g1 = lower rows; g2 = torso+text rows.
def _grp_interval(g, jt):
    j0 = jt * P
    f0 = (jt % 4) * P if jt <= 11 else 0
    if g == 0:
        return (j0, 512 - j0) if jt <= 3 else None
    if g == 1:
        s = j0 if jt <= 3 else f0
        return (512 + s, 512 - s)
    s = j0 if jt <= 3 else f0
    return (1024 + s, 544 - s)


# mask kind per (group, jt): 'T1' | 'T2' | 'TXT' | None
def _grp_mask(g, jt):
    if jt == 12:
        return "TXT" if g in (1, 2) else None
    if g == 0:
        return "T1"
    if g == 1:
        return "T1" if jt <= 3 else "T2"
    return "T1" if jt <= 7 else "T2"


_GRP_ITS = {0: range(0, 4), 1: range(4, 8), 2: range(8, 13)}



def _build_program():
    nc = bacc.Bacc("TRN2", target_bir_lowering=False, debug=False)

    xT_d = nc.dram_tensor("xT", [C, L], F16, kind="ExternalInput")
    wq_d = nc.dram_tensor("wqT", [C, G], F16, kind="ExternalInput")
    wk_d = nc.dram_tensor("wkT", [C, G], F16, kind="ExternalInput")
    wv_d = nc.dram_tensor("wvT", [C, G], F16, kind="ExternalInput")
    wp_d = nc.dram_tensor("wpT", [G, C], F16, kind="ExternalInput")
    bq_d = nc.dram_tensor("bqP", [P, CT], F32, kind="ExternalInput")
    bk_d = nc.dram_tensor("bkP", [P, CT], F32, kind="ExternalInput")
    bv_d = nc.dram_tensor("bvB", [P, G], F32, kind="ExternalInput")
    maskd_d = nc.dram_tensor("maskD", [P, 2, P], F16, kind="ExternalInput")
    maskt_d = nc.dram_tensor("maskTxt", [32, 1024], F16, kind="ExternalInput")
    out_d = nc.dram_tensor("out_part", [L, C], F32, kind="ExternalOutput")

    with tile.TileContext(nc) as tc, ExitStack() as big:
        persist = big.enter_context(tc.tile_pool(name="persist", bufs=1))

        # persistent SBUF tensors
        qT = persist.tile([P, CT, L], F16, name="qT")
        kT = persist.tile([P, CT, L], F16, name="kT")
        v_ones = persist.tile([P, NJT, NH, D + 1], F16, name="v_ones")
        maskD = persist.tile([P, 2, P], F16, name="maskD_sb")
        maskTx = persist.tile([32, 1024], F16, name="maskTx_sb")
        yT = persist.tile([P, CT, L], F16, name="yT")
        wp_sb = persist.tile([P, CT, C], F16, name="wp_sb")
        ones64 = persist.tile([1, D], F16, name="ones64")
        bv_sb = persist.tile([P, G], F32, name="bv_sb")

        nc.sync.dma_start(maskD[:], maskd_d[:])
        nc.sync.dma_start(maskTx[:], maskt_d[:])
        nc.sync.dma_start(wp_sb[:], wp_d.rearrange("(ct p) n -> p ct n", p=P))
        nc.sync.dma_start(bv_sb[:], bv_d[:])
        nc.gpsimd.memset(ones64[:], 1.0)
        nc.gpsimd.memset(v_ones[:], 1.0)

        # ---------- Phase A: projections ----------
        with (
            tc.tile_pool(name="phA", bufs=1) as phA,
            tc.tile_pool(name="psA", bufs=2, space="PSUM") as psA,
        ):
            xT = phA.tile([P, ET, L], F16, name="xT_sb")
            wq_sb = phA.tile([P, ET, G], F16, name="wq_sb")
            wk_sb = phA.tile([P, ET, G], F16, name="wk_sb")
            wv_sb = phA.tile([P, ET, G], F16, name="wv_sb")
            bq_sb = phA.tile([P, CT], F32, name="bq_sb")
            bk_sb = phA.tile([P, CT], F32, name="bk_sb")

            nc.sync.dma_start(xT[:], xT_d.rearrange("(et p) i -> p et i", p=P))
            nc.sync.dma_start(wq_sb[:], wq_d.rearrange("(et p) m -> p et m", p=P))
            nc.sync.dma_start(wk_sb[:], wk_d.rearrange("(et p) m -> p et m", p=P))
            nc.sync.dma_start(wv_sb[:], wv_d.rearrange("(et p) m -> p et m", p=P))
            nc.sync.dma_start(bq_sb[:], bq_d[:])
            nc.sync.dma_start(bk_sb[:], bk_d[:])

            # qT / kT: out[c_tile, i] accumulated over e tiles
            for dst, w_sb, b_sb in ((qT, wq_sb, bq_sb), (kT, wk_sb, bk_sb)):
                for ct in range(CT):
                    for i0, ilen in I_CHUNKS:
                        ps = psA.tile([P, 512], F32, name="ps_qk", tag="ps_qk")
                        for et in range(ET):
                            nc.tensor.matmul(
                                ps[:, :ilen],
                                w_sb[:, et, ct * P : (ct + 1) * P],
                                xT[:, et, i0 : i0 + ilen],
                                start=(et == 0),
                                stop=(et == ET - 1),
                            )
                        nc.vector.tensor_scalar(
                            dst[:, ct, i0 : i0 + ilen],
                            ps[:, :ilen],
                            b_sb[:, ct : ct + 1],
                            None,
                            mybir.AluOpType.add,
                        )

            # v natural layout [i, 384] + bias, into the 65-strided bf16 buffer
            for it in range(NJT):
                il = _jl(it)
                ps = psA.tile([P, G], F32, name="ps_v", tag="ps_v")
                for et in range(ET):
                    nc.tensor.matmul(
                        ps[:il, :],
                        xT[:, et, it * P : it * P + il],
                        wv_sb[:, et, :],
                        start=(et == 0),
                        stop=(et == ET - 1),
                    )
                nc.vector.tensor_tensor(
                    v_ones[:il, it, :, 0:D],
                    ps[:il, :].rearrange("p (h d) -> p h d", h=NH),
                    bv_sb[:il, :].rearrange("p (h d) -> p h d", h=NH),
                    mybir.AluOpType.add,
                )

        # ---------- Phase B+C: attention by row-group, proj interleaved ----------
        with (
            tc.tile_pool(name="phB", bufs=1) as phB,
            tc.tile_pool(name="phC", bufs=3) as phC,
            tc.tile_pool(name="psS", bufs=3, space="PSUM") as psS,
            tc.tile_pool(name="psY", bufs=5, space="PSUM") as psY,
        ):
            for g in range(3):
                jts = [jt for jt in range(NJT) if _grp_interval(g, jt) is not None]
                for h in range(NH):
                    pof = D * (h % 2)
                    ct = h // 2
                    ps_y = {}
                    started = set()
                    for jt in jts:
                        jl = _jl(jt)
                        a, ln = _grp_interval(g, jt)
                        chunks = [(a, min(ln, 512))]
                        if ln > 512:
                            chunks.append((a + 512, ln - 512))
                        for ca, cl in chunks:
                            ps_s = psS.tile([P, 512], F32, name="ps_s", tag="ps_s")
                            nc.tensor.matmul(
                                ps_s[:jl, :cl],
                                kT[pof : pof + D, ct, jt * P : jt * P + jl],
                                qT[pof : pof + D, ct, ca : ca + cl],
                                start=True,
                                stop=True,
                            )
                            pt = phB.tile([P, 512], F16, name="pT", tag="pT", bufs=16)
                            nc.scalar.activation(
                                pt[:jl, :cl],
                                ps_s[:jl, :cl],
                                mybir.ActivationFunctionType.Exp,
                                bias=0.0,
                                scale=SCALE,
                            )
                            mk = _grp_mask(g, jt)
                            if ca == a and mk in ("T1", "T2"):
                                nc.vector.tensor_tensor(
                                    pt[:jl, 0:P],
                                    pt[:jl, 0:P],
                                    maskD[:jl, 0 if mk == "T1" else 1, :],
                                    mybir.AluOpType.mult,
                                )
                            elif ca == a and mk == "TXT":
                                m0 = a - 512
                                nc.vector.tensor_tensor(
                                    pt[:jl, :cl],
                                    pt[:jl, :cl],
                                    maskTx[:jl, m0 : m0 + cl],
                                    mybir.AluOpType.mult,
                                )
                            parts = [(ca, cl, 0)]
                            if ca < 1536 < ca + cl:
                                parts = [
                                    (ca, 1536 - ca, 0),
                                    (1536, ca + cl - 1536, 1536 - ca),
                                ]
                            for pa, pl, poff in parts:
                                ich = _ich_of(pa)
                                off = pa - (0, 512, 1024, 1536)[ich]
                                if ich not in ps_y:
                                    ps_y[ich] = psY.tile(
                                        [D + 1, 512], F32, name=f"ps_y{ich}", tag="ps_y"
                                    )
                                nc.tensor.matmul(
                                    ps_y[ich][:, off : off + pl],
                                    v_ones[:jl, jt, h, :],
                                    pt[:jl, poff : poff + pl],
                                    start=ich not in started,
                                    stop=(jt == jts[-1]),
                                    skip_group_check=True,
                                )
                                started.add(ich)

                    for ich, psy in ps_y.items():
                        i0, ilen = I_CHUNKS[ich]
                        den = phB.tile([1, 512], F16, name="den", tag="den", bufs=4)
                        nc.vector.tensor_copy(den[0:1, :ilen], psy[D : D + 1, :ilen])
                        ps_bc = psS.tile([D, 512], F32, name="ps_bc", tag="ps_s")
                        nc.tensor.matmul(
                            ps_bc[:, :ilen],
                            ones64[0:1, :],
                            den[0:1, :ilen],
                            start=True,
                            stop=True,
                        )
                        rc = phB.tile([D, 512], F32, name="rc", tag="rc", bufs=4)
                        nc.vector.reciprocal_approx_fast(
                            out=rc[:, :ilen], in_=ps_bc[:, :ilen]
                        )
                        nc.vector.tensor_tensor(
                            yT[pof : pof + D, ct, i0 : i0 + ilen],
                            psy[0:D, :ilen],
                            rc[:, :ilen],
                            mybir.AluOpType.mult,
                        )

                # output projection for this group's row tiles
                for it in _GRP_ITS[g]:
                    il = _jl(it)
                    o_sb = phC.tile([P, C], F32, name="o_sb", tag="o_sb")
                    for nch in range(2):
                        ps_o = psS.tile([P, 512], F32, name="ps_o", tag="ps_s")
                        for kt in range(CT):
                            nc.tensor.matmul(
                                ps_o[:il, :384],
                                yT[:, kt, it * P : it * P + il],
                                wp_sb[:, kt, nch * 384 : (nch + 1) * 384],
                                start=(kt == 0),
                                stop=(kt == CT - 1),
                                skip_group_check=True,
                            )
                        nc.any.tensor_copy(
                            o_sb[:il, nch * 384 : (nch + 1) * 384], ps_o[:il, :384]
                        )
                    nc.sync.dma_start(out_d[it * P : it * P + il, :], o_sb[:il, :])

    nc.compile()
    return nc


def _build_mask_np(seg_starts, seg_ends):
    """True = masked. Mirrors reference._build_mask in numpy."""
    ML = 3 * T
    tril = np.tril(np.ones((T, T), dtype=bool))
    sl = np.tril(np.ones((T, T), dtype=bool), -1)
    m = np.zeros((L, L), dtype=bool)
    m[:ML, :ML] = True
    m[0:T, 0:T] = ~tril
    m[T : 2 * T, 0:T] = ~tril
    m[T : 2 * T, T : 2 * T] = ~sl
    m[T : 2 * T, 2 * T : 3 * T] = ~sl
    m[2 * T : 3 * T, 0:T] = ~tril
    m[2 * T : 3 * T, T : 2 * T] = ~tril
    m[2 * T : 3 * T, 2 * T : 3 * T] = ~sl
    m[:ML, ML:] = True
    frames = np.arange(T)[None, :, None]
    allowed = (frames >= seg_starts[:, None, :]) & (frames < seg_ends[:, None, :])
    mask = np.broadcast_to(m[None], (B, L, L)).copy()
    for row0, col_blocks in ((T, (0, 2, 3)), (2 * T, (1, 2, 3))):
        for j in col_blocks:
            c0 = ML + j * N
            mask[:, row0 : row0 + T, c0 : c0 + N] &= ~allowed
    return mask


def get_nc():
    global _NC
    if _NC is None:
        _NC = _build_program()
    return _NC


def make_in_maps(x, Wq, bq, Wk, bk, Wv, bv, Wp, bp, seg_starts, seg_ends):
    mask = _build_mask_np(np.asarray(seg_starts), np.asarray(seg_ends))
    r = np.arange(P)
    maskD = np.empty((P, 2, P), dtype=np.float16)
    maskD[:, 0, :] = (r[:, None] <= r[None, :]).astype(np.float16)  # tril.T
    maskD[:, 1, :] = (r[:, None] < r[None, :]).astype(np.float16)  # strict
    in_maps = []
    for core in range(8):
        b, g = core // 2, core % 2
        gs = slice(g * G, (g + 1) * G)
        allowT = ~mask[b].T  # [j, i]
        maskTx = np.ascontiguousarray(
            allowT[1536:1568, 512:1536].astype(np.float16)
        )
        in_maps.append(
            {
                "xT": np.ascontiguousarray(x[b].T).astype(np.float16),
                "wqT": np.ascontiguousarray(Wq[gs, :].T).astype(np.float16),
                "wkT": np.ascontiguousarray(Wk[gs, :].T).astype(np.float16),
                "wvT": np.ascontiguousarray(Wv[gs, :].T).astype(np.float16),
                "wpT": np.ascontiguousarray(Wp[:, gs].T).astype(np.float16),
                "bqP": np.ascontiguousarray(bq[gs].reshape(CT, P).T),
                "bkP": np.ascontiguousarray(bk[gs].reshape(CT, P).T),
                "bvB": np.broadcast_to(bv[gs], (P, G)).copy(),
                "maskD": maskD,
                "maskTxt": maskTx,
            }
        )
    return in_maps


def kernel(x, Wq, bq, Wk, bk, Wv, bv, Wp, bp, seg_starts, seg_ends, T_motion=None,
           N=None, _trace=False, **_unused):
    x = np.asarray(x, np.float32)
    args = [np.asarray(a, np.float32) for a in (Wq, bq, Wk, bk, Wv, bv, Wp, bp)]
    Wq, bq, Wk, bk, Wv, bv, Wp, bp = args
    nc = get_nc()
    in_maps = make_in_maps(x, Wq, bq, Wk, bk, Wv, bv, Wp, bp, seg_starts, seg_ends)
    res = run_bass_kernel_spmd(nc, in_maps, core_ids=list(range(8)), trace=_trace)
    parts = [r["out_part"] for r in res.results]
    y = np.empty((B, L, C), np.float32)
    for b in range(B):
        y[b] = parts[2 * b] + parts[2 * b + 1] + bp
    if _trace:
        kernel.last_results = res
    return y



# revision 16
# speedup vs baseline: 1.1793x; 1.1793x over previous
"""Cross-conditional GPT2 sparse attention block on 8 Trainium2 NeuronCores.

Sharding: core = (batch b in 0..3) x (head-group g in 0..1, 6 heads each).

Per core, for its (b, head-group):
  qT/kT = (Wq_g @ x_b^T + bq_g)  laid out [d_on_partitions, L]
  v     = x_b @ Wv_g^T + bv_g    natural layout [L, 384], stored interleaved
          with 64 ones columns per head ([L, 6, 128] = v|ones) so att@v also
          broadcasts the softmax denominator across 64 PSUM partitions for
          free; the normalize is then a single DVE divide per (head, i-chunk).
  scores are computed *transposed* (sT[j, i]) so softmax needs no transpose:
  exp is split between the ACT engine (true exp) and the otherwise-idle
  GpSimd engine (Schraudolph fp16 bit-trick exp via tensor_scalar into an
  int16-bitcast view; softmax renormalization cancels its common-mode error).
  The score->exp->mask->att@v chain is software-pipelined with a lag so the
  PE streams matmuls back-to-back (TRN2 ramps 1.2->2.4 GHz only after 3us of
  continuous tensor work). Output projection for group g is interleaved into
  the score stream of group g+1.
Host sums the two per-batch partials (fp16) and adds bp.
"""

import sys

sys.path.insert(0, "/opt/trn_rl_repo")

from contextlib import ExitStack

import ml_dtypes
import numpy as np

import concourse.bacc as bacc
import concourse.bass as bass
import concourse.mybir as mybir
import concourse.tile as tile
from concourse.bass_utils import run_bass_kernel_spmd

# ---- problem constants (hardcoded per spec) ----
B = 4
T = 512
N = 8
C = 768
NHEAD = 12
L = 3 * T + 4 * N  # 1568
P = 128
G = C // 2  # 384 channels per head-group
NH = 6  # heads per core
D = 64  # head dim
ET = C // P  # 6 e-tiles (contraction of x @ W)
CT = G // P  # 3 c-tiles of the group's channels
NJT = (L + P - 1) // P  # 13 j tiles (12x128 + 32)
I_CHUNKS = [(0, 512), (512, 512), (1024, 512), (1536, 32)]
SCALE = 1.0 / 8.0  # 1/sqrt(64)

# Schraudolph fp16 exp: bitcast(int16(x*A + B)) ~= exp(SCALE*x)
EXP_A = float(SCALE * np.log2(np.e) * 1024.0)
EXP_B = 15360.0 - 44.2

F32 = mybir.dt.float32
F16 = mybir.dt.float16
I16 = mybir.dt.int16

LAG = 4  # score->att@v software pipeline depth (in j-tile chunks)

_NC = None  # cached compiled Bass program


def _jl(jt):
    return P if jt < NJT - 1 else L - (NJT - 1) * P  # 128 or 32


def _ich_of(a):
    return 3 if a == 1536 else a // 512


# (group) -> per-jt score interval (a, ln).
# g0 = upper rows (i 0..512), jts 0..3; g1 = lower rows; g2 = torso+text rows.
def _grp_interval(g, jt):
    j0 = jt * P
    f0 = (jt % 4) * P if jt <= 11 else 0
    if g == 0:
        return (j0, 512 - j0) if jt <= 3 else None
    if g == 1:
        s = j0 if jt <= 3 else f0
        return (512 + s, 512 - s)
    s = j0 if jt <= 3 else f0
    return (1024 + s, 544 - s)


# mask kind per (group, jt): 'T1' | 'T2' | 'TXT' | None
def _grp_mask(g, jt):
    if jt == 12:
        return "TXT" if g in (1, 2) else None
    if g == 0:
        return "T1"
    if g == 1:
        return "T1" if jt <= 3 else "T2"
    return "T1" if jt <= 7 else "T2"


_GRP_ITS = {0: range(0, 4), 1: range(4, 8), 2: range(8, 13)}


def _grp_items(g):
    """Pipeline items for group g: (jt, jl, ca, cl, mask, av_parts).
    av_parts: list of (ich, off_in_psy, poff_in_pt, plen)."""
    items = []
    jts = [jt for jt in range(NJT) if _grp_interval(g, jt) is not None]
    for jt in jts:
        jl = _jl(jt)
        a, ln = _grp_interval(g, jt)
        chunks = [(a, min(ln, 512))]
        if ln > 512:
            chunks.append((a + 512, ln - 512))
        for ca, cl in chunks:
            mk = _grp_mask(g, jt) if ca == a else None
            parts = [(ca, cl, 0)]
            if ca < 1536 < ca + cl:
                parts = [(ca, 1536 - ca, 0), (1536, ca + cl - 1536, 1536 - ca)]
            av = []
            for pa, pl, poff in parts:
                ich = _ich_of(pa)
                off = pa - (0, 512, 1024, 1536)[ich]
                av.append((ich, off, poff, pl))
            items.append(
                dict(jt=jt, jl=jl, ca=ca, cl=cl, a=a, mk=mk, av=av, last=(jt == jts[-1]))
            )
    return items


def _build_program():
    nc = bacc.Bacc("TRN2", target_bir_lowering=False, debug=False)

    xT_d = nc.dram_tensor("xT", [C, L], F16, kind="ExternalInput")
    wq_d = nc.dram_tensor("wqT", [C, G], F16, kind="ExternalInput")
    wk_d = nc.dram_tensor("wkT", [C, G], F16, kind="ExternalInput")
    wv_d = nc.dram_tensor("wvT", [C, G], F16, kind="ExternalInput")
    wp_d = nc.dram_tensor("wpT", [G, C], F16, kind="ExternalInput")
    bq_d = nc.dram_tensor("bqP", [P, CT], F32, kind="ExternalInput")
    bk_d = nc.dram_tensor("bkP", [P, CT], F32, kind="ExternalInput")
    bv_d = nc.dram_tensor("bvB", [P, G], F32, kind="ExternalInput")
    maskd_d = nc.dram_tensor("maskD", [P, 2, P], F16, kind="ExternalInput")
    maskt_d = nc.dram_tensor("maskTxt", [32, 1024], F16, kind="ExternalInput")
    out_d = nc.dram_tensor("out_part", [L, C], F16, kind="ExternalOutput")

    with tile.TileContext(nc) as tc, ExitStack() as big:
        persist = big.enter_context(tc.tile_pool(name="persist", bufs=1))

        # persistent SBUF tensors
        qT = persist.tile([P, CT, L], F16, name="qT")
        kT = persist.tile([P, CT, L], F16, name="kT")
        v_ones = persist.tile([P, NJT, NH, 2 * D], F16, name="v_ones")
        maskD = persist.tile([P, 2, P], F16, name="maskD_sb")
        maskTx = persist.tile([32, 1024], F16, name="maskTx_sb")
        yT = persist.tile([P, CT, L], F16, name="yT")
        wp_sb = persist.tile([P, CT, C], F16, name="wp_sb")
        bv_sb = persist.tile([P, G], F32, name="bv_sb")

        nc.gpsimd.memset(v_ones[:], 1.0)

        # ---------- Phase A: projections ----------
        with (
            tc.tile_pool(name="phA", bufs=1) as phA,
            tc.tile_pool(name="psA", bufs=2, space="PSUM") as psA,
        ):
            xT_e = [phA.tile([P, L], F16, name=f"xT_e{et}") for et in range(ET)]
            wq_sb = phA.tile([P, ET, G], F16, name="wq_sb")
            wk_sb = phA.tile([P, ET, G], F16, name="wk_sb")
            wv_sb = phA.tile([P, ET, G], F16, name="wv_sb")
            bq_sb = phA.tile([P, CT], F32, name="bq_sb")
            bk_sb = phA.tile([P, CT], F32, name="bk_sb")

            # x first (matmuls need it first), then q/k weights, then the rest
            for et in range(ET):
                nc.sync.dma_start(xT_e[et][:], xT_d[et * P : (et + 1) * P, :])
            nc.sync.dma_start(wq_sb[:], wq_d.rearrange("(et p) m -> p et m", p=P))
            nc.sync.dma_start(bq_sb[:], bq_d[:])
            nc.sync.dma_start(wk_sb[:], wk_d.rearrange("(et p) m -> p et m", p=P))
            nc.sync.dma_start(bk_sb[:], bk_d[:])
            nc.sync.dma_start(wv_sb[:], wv_d.rearrange("(et p) m -> p et m", p=P))
            nc.sync.dma_start(bv_sb[:], bv_d[:])
            nc.sync.dma_start(maskD[:], maskd_d[:])
            nc.sync.dma_start(maskTx[:], maskt_d[:])
            nc.sync.dma_start(wp_sb[:], wp_d.rearrange("(ct p) n -> p ct n", p=P))

            # qT / kT: out[c_tile, i] accumulated over e tiles
            for dst, w_sb, b_sb in ((qT, wq_sb, bq_sb), (kT, wk_sb, bk_sb)):
                for ct in range(CT):
                    for i0, ilen in I_CHUNKS:
                        ps = psA.tile([P, 512], F32, name="ps_qk", tag="ps_qk", bufs=3)
                        for et in range(ET):
                            nc.tensor.matmul(
                                ps[:, :ilen],
                                w_sb[:, et, ct * P : (ct + 1) * P],
                                xT_e[et][:, i0 : i0 + ilen],
                                start=(et == 0),
                                stop=(et == ET - 1),
                            )
                        nc.vector.tensor_scalar(
                            dst[:, ct, i0 : i0 + ilen],
                            ps[:, :ilen],
                            b_sb[:, ct : ct + 1],
                            None,
                            mybir.AluOpType.add,
                        )

            # v natural layout [i, 384] + bias, into the 128-strided fp16 buffer
            for it in range(NJT):
                il = _jl(it)
                ps = psA.tile([P, G], F32, name="ps_v", tag="ps_v", bufs=3)
                for et in range(ET):
                    nc.tensor.matmul(
                        ps[:il, :],
                        xT_e[et][:, it * P : it * P + il],
                        wv_sb[:, et, :],
                        start=(et == 0),
                        stop=(et == ET - 1),
                    )
                nc.vector.tensor_tensor(
                    v_ones[:il, it, :, D : 2 * D],
                    ps[:il, :].rearrange("p (h d) -> p h d", h=NH),
                    bv_sb[:il, :].rearrange("p (h d) -> p h d", h=NH),
                    mybir.AluOpType.add,
                )

        # ---------- Phase B+C: pipelined attention, proj interleaved ----------
        with (
            tc.tile_pool(name="phB", bufs=1) as phB,
            tc.tile_pool(name="phC", bufs=3) as phC,
            tc.tile_pool(name="psS", bufs=4, space="PSUM") as psS,
            tc.tile_pool(name="psY", bufs=4, space="PSUM") as psY,
        ):
            exp_ctr = 0  # alternates exp between ACT and GpSimd
            cp_ctr = 0  # alternates psum->sbuf copies between ACT and GpSimd

            def issue_score(g, h, it):
                """score matmul + exp + mask for one item; returns pt tile."""
                nonlocal exp_ctr
                pof = D * (h % 2)
                ct = h // 2
                jt, jl, ca, cl = it["jt"], it["jl"], it["ca"], it["cl"]
                ps_s = psS.tile([P, 512], F32, name="ps_s", tag="ps_s")
                nc.tensor.matmul(
                    ps_s[:jl, :cl],
                    kT[pof : pof + D, ct, jt * P : jt * P + jl],
                    qT[pof : pof + D, ct, ca : ca + cl],
                    start=True,
                    stop=True,
                )
                pt = phB.tile([P, 512], F16, name="pT", tag="pT", bufs=2 * LAG + 2)
                # exp split ACT (true exp) vs DVE (Schraudolph bit-trick), 5:3
                if exp_ctr % 8 < 5:
                    nc.scalar.activation(
                        pt[:jl, :cl],
                        ps_s[:jl, :cl],
                        mybir.ActivationFunctionType.Exp,
                        bias=0.0,
                        scale=SCALE,
                    )
                else:
                    nc.vector.tensor_scalar(
                        pt[:jl, :cl].bitcast(I16),
                        ps_s[:jl, :cl],
                        EXP_A,
                        EXP_B,
                        mybir.AluOpType.mult,
                        mybir.AluOpType.add,
                    )
                exp_ctr += 1
                mk = it["mk"]
                if mk in ("T1", "T2"):
                    nc.gpsimd.tensor_tensor(
                        pt[:jl, 0:P],
                        pt[:jl, 0:P],
                        maskD[:jl, 0 if mk == "T1" else 1, :],
                        mybir.AluOpType.mult,
                    )
                elif mk == "TXT":
                    m0 = it["a"] - 512
                    nc.gpsimd.tensor_tensor(
                        pt[:jl, :cl],
                        pt[:jl, :cl],
                        maskTx[:jl, m0 : m0 + cl],
                        mybir.AluOpType.mult,
                    )
                return pt

            def issue_av(g, h, it, pt, ps_y, started):
                jt, jl = it["jt"], it["jl"]
                for ich, off, poff, pl in it["av"]:
                    if ich not in ps_y:
                        ps_y[ich] = psY.tile(
                            [P, 512], F32, name=f"ps_y{ich}", tag="ps_y"
                        )
                    nc.tensor.matmul(
                        ps_y[ich][:, off : off + pl],
                        v_ones[:jl, jt, h, :],
                        pt[:jl, poff : poff + pl],
                        start=ich not in started,
                        stop=it["last"],
                        skip_group_check=True,
                    )
                    started.add(ich)

            def issue_divides(g, h, ps_y):
                pof = D * (h % 2)
                ct = h // 2
                for ich, psy in ps_y.items():
                    i0, ilen = I_CHUNKS[ich]
                    rc = phB.tile([D, 512], F32, name="rc", tag="rc", bufs=4)
                    nc.vector.reciprocal_approx_fast(
                        out=rc[:, :ilen], in_=psy[0:D, :ilen]
                    )
                    nc.vector.tensor_tensor(
                        yT[pof : pof + D, ct, i0 : i0 + ilen],
                        psy[D : 2 * D, :ilen],
                        rc[:, :ilen],
                        mybir.AluOpType.mult,
                    )

            def outproj_groups(g):
                """Output-projection PSUM groups for group g's row tiles."""
                out = []
                for itile in _GRP_ITS[g]:
                    for nch in range(2):
                        out.append((itile, nch))
                return out

            def issue_outproj(itile, nch):
                nonlocal cp_ctr
                il = _jl(itile)
                ps_o = psY.tile([P, 512], F32, name="ps_o", tag="ps_y")
                for kt in range(CT):
                    nc.tensor.matmul(
                        ps_o[:il, :384],
                        yT[:, kt, itile * P : itile * P + il],
                        wp_sb[:, kt, nch * 384 : (nch + 1) * 384],
                        start=(kt == 0),
                        stop=(kt == CT - 1),
                        skip_group_check=True,
                    )
                o_sb = phC.tile([P, 384], F16, name="o_sb", tag="o_sb", bufs=4)
                nc.scalar.copy(o_sb[:il, :], ps_o[:il, :384])
                cp_ctr += 1
                nc.sync.dma_start(
                    out_d[itile * P : itile * P + il, nch * 384 : (nch + 1) * 384],
                    o_sb[:il, :],
                )

            pending_proj = []  # outproj groups of the previous row-group
            for g in range(3):
                items = _grp_items(g)
                for h in range(NH):
                    pipeline = []  # (item, pt) awaiting their av matmul
                    ps_y = {}
                    started = set()
                    for idx, it in enumerate(items):
                        pt = issue_score(g, h, it)
                        pipeline.append((it, pt))
                        if len(pipeline) > LAG:
                            it2, pt2 = pipeline.pop(0)
                            issue_av(g, h, it2, pt2, ps_y, started)
                    for it2, pt2 in pipeline:
                        issue_av(g, h, it2, pt2, ps_y, started)
                    issue_divides(g, h, ps_y)
                    if h == 0:
                        while pending_proj:
                            issue_outproj(*pending_proj.pop(0))
                pending_proj = outproj_groups(g)
            # tail: last group's output projection
            while pending_proj:
                issue_outproj(*pending_proj.pop(0))

    nc.compile()
    return nc


def _build_mask_np(seg_starts, seg_ends):
    """True = masked. Mirrors reference._build_mask in numpy."""
    ML = 3 * T
    tril = np.tril(np.ones((T, T), dtype=bool))
    sl = np.tril(np.ones((T, T), dtype=bool), -1)
    m = np.zeros((L, L), dtype=bool)
    m[:ML, :ML] = True
    m[0:T, 0:T] = ~tril
    m[T : 2 * T, 0:T] = ~tril
    m[T : 2 * T, T : 2 * T] = ~sl
    m[T : 2 * T, 2 * T : 3 * T] = ~sl
    m[2 * T : 3 * T, 0:T] = ~tril
    m[2 * T : 3 * T, T : 2 * T] = ~tril
    m[2 * T : 3 * T, 2 * T : 3 * T] = ~sl
    m[:ML, ML:] = True
    frames = np.arange(T)[None, :, None]
    allowed = (frames >= seg_starts[:, None, :]) & (frames < seg_ends[:, None, :])
    mask = np.broadcast_to(m[None], (B, L, L)).copy()
    for row0, col_blocks in ((T, (0, 2, 3)), (2 * T, (1, 2, 3))):
        for j in col_blocks:
            c0 = ML + j * N
            mask[:, row0 : row0 + T, c0 : c0 + N] &= ~allowed
    return mask


def get_nc():
    global _NC
    if _NC is None:
        _NC = _build_program()
    return _NC


def make_in_maps(x, Wq, bq, Wk, bk, Wv, bv, Wp, bp, seg_starts, seg_ends):
    mask = _build_mask_np(np.asarray(seg_starts), np.asarray(seg_ends))
    r = np.arange(P)
    maskD = np.empty((P, 2, P), dtype=np.float16)
    maskD[:, 0, :] = (r[:, None] <= r[None, :]).astype(np.float16)  # tril.T
    maskD[:, 1, :] = (r[:, None] < r[None, :]).astype(np.float16)  # strict
    in_maps = []
    for core in range(8):
        b, g = core // 2, core % 2
        gs = slice(g * G, (g + 1) * G)
        allowT = ~mask[b].T  # [j, i]
        maskTx = np.ascontiguousarray(
            allowT[1536:1568, 512:1536].astype(np.float16)
        )
        in_maps.append(
            {
                "xT": np.ascontiguousarray(x[b].T).astype(np.float16),
                "wqT": np.ascontiguousarray(Wq[gs, :].T).astype(np.float16),
                "wkT": np.ascontiguousarray(Wk[gs, :].T).astype(np.float16),
                "wvT": np.ascontiguousarray(Wv[gs, :].T).astype(np.float16),
                "wpT": np.ascontiguousarray(Wp[:, gs].T).astype(np.float16),
                "bqP": np.ascontiguousarray(bq[gs].reshape(CT, P).T),
                "bkP": np.ascontiguousarray(bk[gs].reshape(CT, P).T),
                "bvB": np.broadcast_to(bv[gs], (P, G)).copy(),
                "maskD": maskD,
                "maskTxt": maskTx,
            }
        )
    return in_maps


def kernel(x, Wq, bq, Wk, bk, Wv, bv, Wp, bp, seg_starts, seg_ends, T_motion=None,
           N=None, _trace=False, **_unused):
    x = np.asarray(x, np.float32)
    args = [np.asarray(a, np.float32) for a in (Wq, bq, Wk, bk, Wv, bv, Wp, bp)]
    Wq, bq, Wk, bk, Wv, bv, Wp, bp = args
    nc = get_nc()
    in_maps = make_in_maps(x, Wq, bq, Wk, bk, Wv, bv, Wp, bp, seg_starts, seg_ends)
    res = run_bass_kernel_spmd(nc, in_maps, core_ids=list(range(8)), trace=_trace)
    parts = [np.asarray(r["out_part"], np.float32) for r in res.results]
    y = np.empty((B, L, C), np.float32)
    for b in range(B):
        y[b] = parts[2 * b] + parts[2 * b + 1] + bp
    if _trace:
        kernel.last_results = res
    return y


# revision 19
# speedup vs baseline: 1.2020x; 1.0192x over previous
"""Cross-conditional GPT2 sparse attention block on 8 Trainium2 NeuronCores.

Sharding: core = (batch b in 0..3) x (head-group g in 0..1, 6 heads each).

Per core, for its (b, head-group):
  qT/kT = (Wq_g @ x_b^T + bq_g)  laid out [d_on_partitions, L]
  v     = x_b @ Wv_g^T + bv_g    natural layout [L, 384], stored interleaved
          with 64 ones columns per head ([L, 6, 128] = v|ones) so att@v also
          broadcasts the softmax denominator across 64 PSUM partitions for
          free; the normalize is then a single DVE divide per (head, i-chunk).
  scores are computed *transposed* (sT[j, i]) so softmax needs no transpose:
  exp is split between the ACT engine (true exp) and the otherwise-idle
  GpSimd engine (Schraudolph fp16 bit-trick exp via tensor_scalar into an
  int16-bitcast view; softmax renormalization cancels its common-mode error).
  The score->exp->mask->att@v chain is software-pipelined with a lag so the
  PE streams matmuls back-to-back (TRN2 ramps 1.2->2.4 GHz only after 3us of
  continuous tensor work). Output projection for group g is interleaved into
  the score stream of group g+1.
Host sums the two per-batch partials (fp16) and adds bp.
"""

import sys

sys.path.insert(0, "/opt/trn_rl_repo")

from contextlib import ExitStack

import ml_dtypes
import numpy as np

import concourse.bacc as bacc
import concourse.bass as bass
import concourse.mybir as mybir
import concourse.tile as tile
from concourse.bass_utils import run_bass_kernel_spmd

# ---- problem constants (hardcoded per spec) ----
B = 4
T = 512
N = 8
C = 768
NHEAD = 12
L = 3 * T + 4 * N  # 1568
P = 128
G = C // 2  # 384 channels per head-group
NH = 6  # heads per core
D = 64  # head dim
ET = C // P  # 6 e-tiles (contraction of x @ W)
CT = G // P  # 3 c-tiles of the group's channels
NJT = (L + P - 1) // P  # 13 j tiles (12x128 + 32)
I_CHUNKS = [(0, 512), (512, 512), (1024, 512), (1536, 32)]
SCALE = 1.0 / 8.0  # 1/sqrt(64)

# Schraudolph fp16 exp: bitcast(int16(x*A + B)) ~= exp(SCALE*x)
EXP_A = float(SCALE * np.log2(np.e) * 1024.0)
EXP_B = 15360.0 - 44.2

F32 = mybir.dt.float32
F16 = mybir.dt.float16
I16 = mybir.dt.int16

LAG = 4  # score->att@v software pipeline depth (in j-tile chunks)

_NC = None  # cached compiled Bass program


def _jl(jt):
    return P if jt < NJT - 1 else L - (NJT - 1) * P  # 128 or 32


def _ich_of(a):
    return 3 if a == 1536 else a // 512


# (group) -> per-jt score interval (a, ln).
# g0 = upper rows (i 0..512), jts 0..3; g1 = lower rows; g2 = torso+text rows.
def _grp_interval(g, jt):
    j0 = jt * P
    f0 = (jt % 4) * P if jt <= 11 else 0
    if g == 0:
        return (j0, 512 - j0) if jt <= 3 else None
    if g == 1:
        s = j0 if jt <= 3 else f0
        return (512 + s, 512 - s)
    s = j0 if jt <= 3 else f0
    return (1024 + s, 544 - s)


# mask kind per (group, jt): 'T1' | 'T2' | 'TXT' | None
def _grp_mask(g, jt):
    if jt == 12:
        return "TXT" if g in (1, 2) else None
    if g == 0:
        return "T1"
    if g == 1:
        return "T1" if jt <= 3 else "T2"
    return "T1" if jt <= 7 else "T2"


_GRP_ITS = {0: range(0, 4), 1: range(4, 8), 2: range(8, 13)}


def _grp_items(g):
    """Pipeline items for group g: (jt, jl, ca, cl, mask, av_parts).
    av_parts: list of (ich, off_in_psy, poff_in_pt, plen)."""
    items = []
    jts = [jt for jt in range(NJT) if _grp_interval(g, jt) is not None]
    for jt in jts:
        jl = _jl(jt)
        a, ln = _grp_interval(g, jt)
        chunks = [(a, min(ln, 512))]
        if ln > 512:
            chunks.append((a + 512, ln - 512))
        for ca, cl in chunks:
            mk = _grp_mask(g, jt) if ca == a else None
            parts = [(ca, cl, 0)]
            if ca < 1536 < ca + cl:
                parts = [(ca, 1536 - ca, 0), (1536, ca + cl - 1536, 1536 - ca)]
            av = []
            for pa, pl, poff in parts:
                ich = _ich_of(pa)
                off = pa - (0, 512, 1024, 1536)[ich]
                av.append((ich, off, poff, pl))
            items.append(
                dict(jt=jt, jl=jl, ca=ca, cl=cl, a=a, mk=mk, av=av, last=(jt == jts[-1]))
            )
    return items


def _build_program():
    nc = bacc.Bacc("TRN2", target_bir_lowering=False, debug=False)

    xT_d = nc.dram_tensor("xT", [C, L], F16, kind="ExternalInput")
    wq_d = nc.dram_tensor("wqT", [C, G], F16, kind="ExternalInput")
    wk_d = nc.dram_tensor("wkT", [C, G], F16, kind="ExternalInput")
    wv_d = nc.dram_tensor("wvT", [C, G], F16, kind="ExternalInput")
    wp_d = nc.dram_tensor("wpT", [G, C], F16, kind="ExternalInput")
    bq_d = nc.dram_tensor("bqP", [P, CT], F32, kind="ExternalInput")
    bk_d = nc.dram_tensor("bkP", [P, CT], F32, kind="ExternalInput")
    bv_d = nc.dram_tensor("bvB", [P, G], F32, kind="ExternalInput")
    maskd_d = nc.dram_tensor("maskD", [P, 2, P], F16, kind="ExternalInput")
    maskt_d = nc.dram_tensor("maskTxt", [32, 1024], F16, kind="ExternalInput")
    out_d = nc.dram_tensor("out_part", [L, C], F16, kind="ExternalOutput")

    with tile.TileContext(nc) as tc, ExitStack() as big:
        persist = big.enter_context(tc.tile_pool(name="persist", bufs=1))

        # persistent SBUF tensors
        qT = persist.tile([P, CT, L], F16, name="qT")
        kT = persist.tile([P, CT, L], F16, name="kT")
        v_ones = persist.tile([P, NJT, NH, 2 * D], F16, name="v_ones")
        maskD = persist.tile([P, 2, P], F16, name="maskD_sb")
        maskTx = persist.tile([32, 1024], F16, name="maskTx_sb")
        yT = persist.tile([P, CT, L], F16, name="yT")
        wp_sb = persist.tile([P, CT, C], F16, name="wp_sb")
        bv_sb = persist.tile([P, G], F32, name="bv_sb")

        nc.gpsimd.memset(v_ones[:], 1.0)

        # ---------- Phase A: projections ----------
        with (
            tc.tile_pool(name="phA", bufs=1) as phA,
            tc.tile_pool(name="psA", bufs=2, space="PSUM") as psA,
        ):
            xT_e = [phA.tile([P, L], F16, name=f"xT_e{et}") for et in range(ET)]
            wq_sb = phA.tile([P, ET, G], F16, name="wq_sb")
            wk_sb = phA.tile([P, ET, G], F16, name="wk_sb")
            wv_sb = phA.tile([P, ET, G], F16, name="wv_sb")
            bq_sb = phA.tile([P, CT], F32, name="bq_sb")
            bk_sb = phA.tile([P, CT], F32, name="bk_sb")

            # weights for the first matmuls first, then x, then the rest
            nc.sync.dma_start(wq_sb[:], wq_d.rearrange("(et p) m -> p et m", p=P))
            nc.sync.dma_start(bq_sb[:], bq_d[:])
            for et in range(ET):
                nc.sync.dma_start(xT_e[et][:], xT_d[et * P : (et + 1) * P, :])
            nc.sync.dma_start(wk_sb[:], wk_d.rearrange("(et p) m -> p et m", p=P))
            nc.sync.dma_start(bk_sb[:], bk_d[:])
            nc.sync.dma_start(wv_sb[:], wv_d.rearrange("(et p) m -> p et m", p=P))
            nc.sync.dma_start(bv_sb[:], bv_d[:])
            nc.sync.dma_start(maskD[:], maskd_d[:])
            nc.sync.dma_start(maskTx[:], maskt_d[:])
            nc.sync.dma_start(wp_sb[:], wp_d.rearrange("(ct p) n -> p ct n", p=P))

            # qT / kT: out[c_tile, i] accumulated over e tiles. Two i-chunk
            # accumulation chains run interleaved so consecutive PE matmuls
            # target different PSUM banks (same-bank back-to-back accumulation
            # stalls the PE pipeline ~180ns per matmul).
            for dst, w_sb, b_sb in ((qT, wq_sb, bq_sb), (kT, wk_sb, bk_sb)):
                for ct in range(CT):
                    for pair in ((0, 1), (2, 3)):
                        pss = {}
                        for et in range(ET):
                            for ic in pair:
                                i0, ilen = I_CHUNKS[ic]
                                if ic not in pss:
                                    pss[ic] = psA.tile(
                                        [P, 512], F32, name="ps_qk", tag="ps_qk", bufs=4
                                    )
                                nc.tensor.matmul(
                                    pss[ic][:, :ilen],
                                    w_sb[:, et, ct * P : (ct + 1) * P],
                                    xT_e[et][:, i0 : i0 + ilen],
                                    start=(et == 0),
                                    stop=(et == ET - 1),
                                    skip_group_check=True,
                                )
                        for ic in pair:
                            i0, ilen = I_CHUNKS[ic]
                            nc.vector.tensor_scalar(
                                dst[:, ct, i0 : i0 + ilen],
                                pss[ic][:, :ilen],
                                b_sb[:, ct : ct + 1],
                                None,
                                mybir.AluOpType.add,
                            )

            # v natural layout [i, 384] + bias, into the 128-strided fp16
            # buffer; it-pairs interleaved for the same bank-alternation reason
            for it0 in range(0, NJT, 2):
                its = [it for it in (it0, it0 + 1) if it < NJT]
                pss = {}
                for et in range(ET):
                    for it in its:
                        il = _jl(it)
                        if it not in pss:
                            pss[it] = psA.tile(
                                [P, G], F32, name="ps_v", tag="ps_v", bufs=4
                            )
                        nc.tensor.matmul(
                            pss[it][:il, :],
                            xT_e[et][:, it * P : it * P + il],
                            wv_sb[:, et, :],
                            start=(et == 0),
                            stop=(et == ET - 1),
                            skip_group_check=True,
                        )
                for it in its:
                    il = _jl(it)
                    nc.vector.tensor_tensor(
                        v_ones[:il, it, :, D : 2 * D],
                        pss[it][:il, :].rearrange("p (h d) -> p h d", h=NH),
                        bv_sb[:il, :].rearrange("p (h d) -> p h d", h=NH),
                        mybir.AluOpType.add,
                    )

        # ---------- Phase B+C: pipelined attention, proj interleaved ----------
        with (
            tc.tile_pool(name="phB", bufs=1) as phB,
            tc.tile_pool(name="phC", bufs=3) as phC,
            tc.tile_pool(name="psS", bufs=4, space="PSUM") as psS,
            tc.tile_pool(name="psY", bufs=4, space="PSUM") as psY,
        ):
            exp_ctr = 0  # alternates exp between ACT and GpSimd
            cp_ctr = 0  # alternates psum->sbuf copies between ACT and GpSimd

            def issue_score(g, h, it):
                """score matmul + exp + mask for one item; returns pt tile."""
                nonlocal exp_ctr
                pof = D * (h % 2)
                ct = h // 2
                jt, jl, ca, cl = it["jt"], it["jl"], it["ca"], it["cl"]
                ps_s = psS.tile([P, 512], F32, name="ps_s", tag="ps_s")
                nc.tensor.matmul(
                    ps_s[:jl, :cl],
                    kT[pof : pof + D, ct, jt * P : jt * P + jl],
                    qT[pof : pof + D, ct, ca : ca + cl],
                    start=True,
                    stop=True,
                )
                pt = phB.tile([P, 512], F16, name="pT", tag="pT", bufs=2 * LAG + 2)
                # exp split ACT (true exp) vs DVE (Schraudolph bit-trick), 5:3
                if exp_ctr % 8 < 5:
                    nc.scalar.activation(
                        pt[:jl, :cl],
                        ps_s[:jl, :cl],
                        mybir.ActivationFunctionType.Exp,
                        bias=0.0,
                        scale=SCALE,
                    )
                else:
                    nc.vector.tensor_scalar(
                        pt[:jl, :cl].bitcast(I16),
                        ps_s[:jl, :cl],
                        EXP_A,
                        EXP_B,
                        mybir.AluOpType.mult,
                        mybir.AluOpType.add,
                    )
                exp_ctr += 1
                mk = it["mk"]
                if mk in ("T1", "T2"):
                    nc.gpsimd.tensor_tensor(
                        pt[:jl, 0:P],
                        pt[:jl, 0:P],
                        maskD[:jl, 0 if mk == "T1" else 1, :],
                        mybir.AluOpType.mult,
                    )
                elif mk == "TXT":
                    m0 = it["a"] - 512
                    nc.gpsimd.tensor_tensor(
                        pt[:jl, :cl],
                        pt[:jl, :cl],
                        maskTx[:jl, m0 : m0 + cl],
                        mybir.AluOpType.mult,
                    )
                return pt

            def issue_av(g, h, it, pt, ps_y, started):
                jt, jl = it["jt"], it["jl"]
                for ich, off, poff, pl in it["av"]:
                    if ich not in ps_y:
                        ps_y[ich] = psY.tile(
                            [P, 512], F32, name=f"ps_y{ich}", tag="ps_y"
                        )
                    nc.tensor.matmul(
                        ps_y[ich][:, off : off + pl],
                        v_ones[:jl, jt, h, :],
                        pt[:jl, poff : poff + pl],
                        start=ich not in started,
                        stop=it["last"],
                        skip_group_check=True,
                    )
                    started.add(ich)

            def issue_divides(g, h, ps_y):
                pof = D * (h % 2)
                ct = h // 2
                for ich, psy in ps_y.items():
                    i0, ilen = I_CHUNKS[ich]
                    rc = phB.tile([D, 512], F32, name="rc", tag="rc", bufs=4)
                    nc.vector.reciprocal_approx_fast(
                        out=rc[:, :ilen], in_=psy[0:D, :ilen]
                    )
                    nc.vector.tensor_tensor(
                        yT[pof : pof + D, ct, i0 : i0 + ilen],
                        psy[D : 2 * D, :ilen],
                        rc[:, :ilen],
                        mybir.AluOpType.mult,
                    )

            def outproj_groups(g):
                """Output-projection row tiles for group g."""
                return list(_GRP_ITS[g])

            def issue_outproj(itile):
                il = _jl(itile)
                ps_o = {
                    nch: psY.tile([P, 512], F32, name="ps_o", tag="ps_y")
                    for nch in range(2)
                }
                for kt in range(CT):
                    for nch in range(2):
                        nc.tensor.matmul(
                            ps_o[nch][:il, :384],
                            yT[:, kt, itile * P : itile * P + il],
                            wp_sb[:, kt, nch * 384 : (nch + 1) * 384],
                            start=(kt == 0),
                            stop=(kt == CT - 1),
                            skip_group_check=True,
                        )
                for nch in range(2):
                    o_sb = phC.tile([P, 384], F16, name="o_sb", tag="o_sb", bufs=4)
                    nc.scalar.copy(o_sb[:il, :], ps_o[nch][:il, :384])
                    nc.sync.dma_start(
                        out_d[itile * P : itile * P + il, nch * 384 : (nch + 1) * 384],
                        o_sb[:il, :],
                    )

            pending_proj = []  # outproj groups of the previous row-group
            for g in range(3):
                items = _grp_items(g)
                for h in range(NH):
                    pipeline = []  # (item, pt) awaiting their av matmul
                    ps_y = {}
                    started = set()
                    for idx, it in enumerate(items):
                        pt = issue_score(g, h, it)
                        pipeline.append((it, pt))
                        if len(pipeline) > LAG:
                            it2, pt2 = pipeline.pop(0)
                            issue_av(g, h, it2, pt2, ps_y, started)
                    for it2, pt2 in pipeline:
                        issue_av(g, h, it2, pt2, ps_y, started)
                    issue_divides(g, h, ps_y)
                    if h == 0:
                        while pending_proj:
                            issue_outproj(pending_proj.pop(0))
                pending_proj = outproj_groups(g)
            # tail: last group's output projection
            while pending_proj:
                issue_outproj(pending_proj.pop(0))

    nc.compile()
    return nc


def _build_mask_np(seg_starts, seg_ends):
    """True = masked. Mirrors reference._build_mask in numpy."""
    ML = 3 * T
    tril = np.tril(np.ones((T, T), dtype=bool))
    sl = np.tril(np.ones((T, T), dtype=bool), -1)
    m = np.zeros((L, L), dtype=bool)
    m[:ML, :ML] = True
    m[0:T, 0:T] = ~tril
    m[T : 2 * T, 0:T] = ~tril
    m[T : 2 * T, T : 2 * T] = ~sl
    m[T : 2 * T, 2 * T : 3 * T] = ~sl
    m[2 * T : 3 * T, 0:T] = ~tril
    m[2 * T : 3 * T, T : 2 * T] = ~tril
    m[2 * T : 3 * T, 2 * T : 3 * T] = ~sl
    m[:ML, ML:] = True
    frames = np.arange(T)[None, :, None]
    allowed = (frames >= seg_starts[:, None, :]) & (frames < seg_ends[:, None, :])
    mask = np.broadcast_to(m[None], (B, L, L)).copy()
    for row0, col_blocks in ((T, (0, 2, 3)), (2 * T, (1, 2, 3))):
        for j in col_blocks:
            c0 = ML + j * N
            mask[:, row0 : row0 + T, c0 : c0 + N] &= ~allowed
    return mask


def get_nc():
    global _NC
    if _NC is None:
        _NC = _build_program()
    return _NC


def make_in_maps(x, Wq, bq, Wk, bk, Wv, bv, Wp, bp, seg_starts, seg_ends):
    mask = _build_mask_np(np.asarray(seg_starts), np.asarray(seg_ends))
    r = np.arange(P)
    maskD = np.empty((P, 2, P), dtype=np.float16)
    maskD[:, 0, :] = (r[:, None] <= r[None, :]).astype(np.float16)  # tril.T
    maskD[:, 1, :] = (r[:, None] < r[None, :]).astype(np.float16)  # strict
    in_maps = []
    for core in range(8):
        b, g = core // 2, core % 2
        gs = slice(g * G, (g + 1) * G)
        allowT = ~mask[b].T  # [j, i]
        maskTx = np.ascontiguousarray(
            allowT[1536:1568, 512:1536].astype(np.float16)
        )
        in_maps.append(
            {
                "xT": np.ascontiguousarray(x[b].T).astype(np.float16),
                "wqT": np.ascontiguousarray(Wq[gs, :].T).astype(np.float16),
                "wkT": np.ascontiguousarray(Wk[gs, :].T).astype(np.float16),
                "wvT": np.ascontiguousarray(Wv[gs, :].T).astype(np.float16),
                "wpT": np.ascontiguousarray(Wp[:, gs].T).astype(np.float16),
                "bqP": np.ascontiguousarray(bq[gs].reshape(CT, P).T),
                "bkP": np.ascontiguousarray(bk[gs].reshape(CT, P).T),
                "bvB": np.broadcast_to(bv[gs], (P, G)).copy(),
                "maskD": maskD,
                "maskTxt": maskTx,
            }
        )
    return in_maps


def kernel(x, Wq, bq, Wk, bk, Wv, bv, Wp, bp, seg_starts, seg_ends, T_motion=None,
           N=None, _trace=False, **_unused):
    x = np.asarray(x, np.float32)
    args = [np.asarray(a, np.float32) for a in (Wq, bq, Wk, bk, Wv, bv, Wp, bp)]
    Wq, bq, Wk, bk, Wv, bv, Wp, bp = args
    nc = get_nc()
    in_maps = make_in_maps(x, Wq, bq, Wk, bk, Wv, bv, Wp, bp, seg_starts, seg_ends)
    res = run_bass_kernel_spmd(nc, in_maps, core_ids=list(range(8)), trace=_trace)
    parts = [np.asarray(r["out_part"], np.float32) for r in res.results]
    y = np.empty((B, L, C), np.float32)
    for b in range(B):
        y[b] = parts[2 * b] + parts[2 * b + 1] + bp
    if _trace:
        kernel.last_results = res
    return y


# revision 23
# speedup vs baseline: 1.2209x; 1.0157x over previous
"""Cross-conditional GPT2 sparse attention block on 8 Trainium2 NeuronCores.

Sharding: core = (batch b in 0..3) x (head-group g in 0..1, 6 heads each).

Per core, for its (b, head-group):
  qT/kT = (Wq_g @ x_b^T + bq_g)  laid out [d_on_partitions, L]
  v     = x_b @ Wv_g^T + bv_g    natural layout [L, 384], stored interleaved
          with 64 ones columns per head ([L, 6, 128] = v|ones) so att@v also
          broadcasts the softmax denominator across 64 PSUM partitions for
          free; the normalize is then a single DVE divide per (head, i-chunk).
  scores are computed *transposed* (sT[j, i]) so softmax needs no transpose:
  exp is split between the ACT engine (true exp) and the otherwise-idle
  GpSimd engine (Schraudolph fp16 bit-trick exp via tensor_scalar into an
  int16-bitcast view; softmax renormalization cancels its common-mode error).
  The score->exp->mask->att@v chain is software-pipelined with a lag so the
  PE streams matmuls back-to-back (TRN2 ramps 1.2->2.4 GHz only after 3us of
  continuous tensor work). Output projection for group g is interleaved into
  the score stream of group g+1.
Host sums the two per-batch partials (fp16) and adds bp.
"""

import sys

sys.path.insert(0, "/opt/trn_rl_repo")

from contextlib import ExitStack

import ml_dtypes
import numpy as np

import concourse.bacc as bacc
import concourse.bass as bass
import concourse.mybir as mybir
import concourse.tile as tile
from concourse.bass_utils import run_bass_kernel_spmd

# ---- problem constants (hardcoded per spec) ----
B = 4
T = 512
N = 8
C = 768
NHEAD = 12
L = 3 * T + 4 * N  # 1568
P = 128
G = C // 2  # 384 channels per head-group
NH = 6  # heads per core
D = 64  # head dim
ET = C // P  # 6 e-tiles (contraction of x @ W)
CT = G // P  # 3 c-tiles of the group's channels
NJT = (L + P - 1) // P  # 13 j tiles (12x128 + 32)
I_CHUNKS = [(0, 512), (512, 512), (1024, 512), (1536, 32)]
SCALE = 1.0 / 8.0  # 1/sqrt(64)

WSC = 64.0  # fp8 weight pre-scale (W*64 avoids e4m3 subnormals)
QSC = 4.0  # fp8 q/k quantization scale
ESC = SCALE / (QSC * QSC)  # exp scale absorbing q8*k8 = 16*q*k

# Schraudolph fp16 exp: bitcast(int16(x*A + B)) ~= exp(ESC*x)
EXP_A = float(ESC * np.log2(np.e) * 1024.0)
EXP_B = 15360.0 - 44.2

F32 = mybir.dt.float32
F16 = mybir.dt.float16
F8 = mybir.dt.float8e4
I16 = mybir.dt.int16
DR = mybir.MatmulPerfMode.DoubleRow

LAG = 4  # score->att@v software pipeline depth (in j-tile chunks)

_NC = None  # cached compiled Bass program


def _jl(jt):
    return P if jt < NJT - 1 else L - (NJT - 1) * P  # 128 or 32


def _ich_of(a):
    return 3 if a == 1536 else a // 512


# (group) -> per-jt score interval (a, ln).
# g0 = upper rows (i 0..512), jts 0..3; g1 = lower rows; g2 = torso+text rows.
def _grp_interval(g, jt):
    j0 = jt * P
    f0 = (jt % 4) * P if jt <= 11 else 0
    if g == 0:
        return (j0, 512 - j0) if jt <= 3 else None
    if g == 1:
        s = j0 if jt <= 3 else f0
        return (512 + s, 512 - s)
    s = j0 if jt <= 3 else f0
    return (1024 + s, 544 - s)


# mask kind per (group, jt): 'T1' | 'T2' | 'TXT' | None
def _grp_mask(g, jt):
    if jt == 12:
        return "TXT" if g in (1, 2) else None
    if g == 0:
        return "T1"
    if g == 1:
        return "T1" if jt <= 3 else "T2"
    return "T1" if jt <= 7 else "T2"


_GRP_ITS = {0: range(0, 4), 1: range(4, 8), 2: range(8, 13)}


def _grp_items(g):
    """Pipeline items for group g: (jt, jl, ca, cl, mask, av_parts).
    av_parts: list of (ich, off_in_psy, poff_in_pt, plen)."""
    items = []
    jts = [jt for jt in range(NJT) if _grp_interval(g, jt) is not None]
    for jt in jts:
        jl = _jl(jt)
        a, ln = _grp_interval(g, jt)
        chunks = [(a, min(ln, 512))]
        if ln > 512:
            chunks.append((a + 512, ln - 512))
        for ca, cl in chunks:
            mk = _grp_mask(g, jt) if ca == a else None
            parts = [(ca, cl, 0)]
            if ca < 1536 < ca + cl:
                parts = [(ca, 1536 - ca, 0), (1536, ca + cl - 1536, 1536 - ca)]
            av = []
            for pa, pl, poff in parts:
                ich = _ich_of(pa)
                off = pa - (0, 512, 1024, 1536)[ich]
                av.append((ich, off, poff, pl))
            items.append(
                dict(jt=jt, jl=jl, ca=ca, cl=cl, a=a, mk=mk, av=av, last=(jt == jts[-1]))
            )
    return items


def _build_program():
    nc = bacc.Bacc("TRN2", target_bir_lowering=False, debug=False)

    x8_d = nc.dram_tensor("x8", [P, 3, 2, L], F8, kind="ExternalInput")
    xT_d = nc.dram_tensor("xT", [C, L], F16, kind="ExternalInput")
    wq_d = nc.dram_tensor("w8q", [P, 3, 2, G], F8, kind="ExternalInput")
    wk_d = nc.dram_tensor("w8k", [P, 3, 2, G], F8, kind="ExternalInput")
    wv_d = nc.dram_tensor("wvT", [C, G], F16, kind="ExternalInput")
    wp_d = nc.dram_tensor("wpT", [G, C], F16, kind="ExternalInput")
    bq_d = nc.dram_tensor("bqP", [P, CT], F32, kind="ExternalInput")
    bk_d = nc.dram_tensor("bkP", [P, CT], F32, kind="ExternalInput")
    bv_d = nc.dram_tensor("bvB", [P, G], F32, kind="ExternalInput")
    maskd_d = nc.dram_tensor("maskD", [P, 2, P], F16, kind="ExternalInput")
    maskt_d = nc.dram_tensor("maskTxt", [32, 1024], F16, kind="ExternalInput")
    out_d = nc.dram_tensor("out_part", [L, C], F16, kind="ExternalOutput")

    with tile.TileContext(nc) as tc, ExitStack() as big:
        persist = big.enter_context(tc.tile_pool(name="persist", bufs=1))

        # persistent SBUF tensors
        q8r = persist.tile([P, CT, L], F8, name="q8r")
        k8r = persist.tile([P, CT, L], F8, name="k8r")
        q8s = persist.tile([P, 2, 2, L], F8, name="q8s")
        k8s = persist.tile([P, 2, 2, L], F8, name="k8s")
        v_ones = persist.tile([P, NJT, NH, 2 * D], F16, name="v_ones")
        maskD = persist.tile([P, 2, P], F16, name="maskD_sb")
        maskTx = persist.tile([32, 1024], F16, name="maskTx_sb")
        yT = persist.tile([P, CT, L], F16, name="yT")
        wp_sb = persist.tile([P, CT, C], F16, name="wp_sb")
        bv_sb = persist.tile([P, G], F32, name="bv_sb")

        nc.gpsimd.memset(v_ones[:], 1.0)

        # ---------- Phase A: projections ----------
        with (
            tc.tile_pool(name="phA", bufs=1) as phA,
            tc.tile_pool(name="psA", bufs=2, space="PSUM") as psA,
        ):
            x8_e = [phA.tile([P, 2, L], F8, name=f"x8_e{ep}") for ep in range(3)]
            xT_e = [phA.tile([P, L], F16, name=f"xT_e{et}") for et in range(ET)]
            wq_sb = phA.tile([P, 3, 2, G], F8, name="wq_sb")
            wk_sb = phA.tile([P, 3, 2, G], F8, name="wk_sb")
            wv_sb = phA.tile([P, ET, G], F16, name="wv_sb")
            bq_sb = phA.tile([P, CT], F32, name="bq_sb")
            bk_sb = phA.tile([P, CT], F32, name="bk_sb")

            # weights for the first matmuls first, then x, then the rest
            nc.sync.dma_start(wq_sb[:], wq_d[:])
            nc.sync.dma_start(bq_sb[:], bq_d[:])
            for ep in range(3):
                nc.sync.dma_start(x8_e[ep][:], x8_d[:, ep])
            nc.sync.dma_start(wk_sb[:], wk_d[:])
            nc.sync.dma_start(bk_sb[:], bk_d[:])
            nc.sync.dma_start(wv_sb[:], wv_d.rearrange("(et p) m -> p et m", p=P))
            nc.sync.dma_start(bv_sb[:], bv_d[:])
            for et in range(ET):
                nc.sync.dma_start(xT_e[et][:], xT_d[et * P : (et + 1) * P, :])
            nc.sync.dma_start(maskD[:], maskd_d[:])
            nc.sync.dma_start(maskTx[:], maskt_d[:])
            nc.sync.dma_start(wp_sb[:], wp_d.rearrange("(ct p) n -> p ct n", p=P))

            # qT / kT: out[c_tile, i] accumulated over e tiles. Two i-chunk
            # accumulation chains run interleaved so consecutive PE matmuls
            # target different PSUM banks (same-bank back-to-back accumulation
            # stalls the PE pipeline ~180ns per matmul).
            for dst, w_sb, b_sb in ((q8r, wq_sb, bq_sb), (k8r, wk_sb, bk_sb)):
                for ct in range(CT):
                    for pair in ((0, 1), (2, 3)):
                        pss = {}
                        for ep in range(3):
                            for ic in pair:
                                i0, ilen = I_CHUNKS[ic]
                                if ic not in pss:
                                    pss[ic] = psA.tile(
                                        [P, 512], F32, name="ps_qk", tag="ps_qk", bufs=4
                                    )
                                nc.tensor.matmul(
                                    pss[ic][:, :ilen],
                                    w_sb[:, ep, :, ct * P : (ct + 1) * P],
                                    x8_e[ep][:, :, i0 : i0 + ilen],
                                    start=(ep == 0),
                                    stop=(ep == 2),
                                    perf_mode=DR,
                                    skip_group_check=True,
                                )
                        for ic in pair:
                            i0, ilen = I_CHUNKS[ic]
                            # q8 = (psum/WSC + b) * QSC, cast to fp8
                            nc.vector.tensor_scalar(
                                dst[:, ct, i0 : i0 + ilen],
                                pss[ic][:, :ilen],
                                QSC / WSC,
                                b_sb[:, ct : ct + 1],
                                mybir.AluOpType.mult,
                                mybir.AluOpType.add,
                            )

            # v natural layout [i, 384] + bias, into the 128-strided fp16
            # buffer; it-pairs interleaved for the same bank-alternation reason
            for it0 in range(0, NJT, 2):
                its = [it for it in (it0, it0 + 1) if it < NJT]
                pss = {}
                for et in range(ET):
                    for it in its:
                        il = _jl(it)
                        if it not in pss:
                            pss[it] = psA.tile(
                                [P, G], F32, name="ps_v", tag="ps_v", bufs=4
                            )
                        nc.tensor.matmul(
                            pss[it][:il, :],
                            xT_e[et][:, it * P : it * P + il],
                            wv_sb[:, et, :],
                            start=(et == 0),
                            stop=(et == ET - 1),
                            skip_group_check=True,
                        )
                for it in its:
                    il = _jl(it)
                    nc.vector.tensor_tensor(
                        v_ones[:il, it, :, D : 2 * D],
                        pss[it][:il, :].rearrange("p (h d) -> p h d", h=NH),
                        bv_sb[:il, :].rearrange("p (h d) -> p h d", h=NH),
                        mybir.AluOpType.add,
                    )

            # shuffle q8r/k8r into the DoubleRow d-split layout
            # (matmul base partition must be 0/32/64 -> 3 heads per span):
            # dst[32*(h%3)+p, h//3, s, i] = src[64*(h%2)+32*s+p, h//2, i]
            for dst_s, src_r in ((q8s, q8r), (k8s, k8r)):
                for h in range(NH):
                    for sdim in range(2):
                        nc.sync.dma_start(
                            dst_s[32 * (h % 3) : 32 * (h % 3) + 32, h // 3, sdim, :],
                            src_r[64 * (h % 2) + 32 * sdim : 64 * (h % 2) + 32 * sdim + 32, h // 2, :],
                        )

        # ---------- Phase B+C: pipelined attention, proj interleaved ----------
        with (
            tc.tile_pool(name="phB", bufs=1) as phB,
            tc.tile_pool(name="phC", bufs=3) as phC,
            tc.tile_pool(name="psS", bufs=4, space="PSUM") as psS,
            tc.tile_pool(name="psY", bufs=4, space="PSUM") as psY,
        ):
            exp_ctr = 0  # alternates exp between ACT and GpSimd
            cp_ctr = 0  # alternates psum->sbuf copies between ACT and GpSimd

            def issue_score(g, h, it):
                """score matmul + exp + mask for one item; returns pt tile."""
                nonlocal exp_ctr
                pof = D * (h % 2)
                ct = h // 2
                jt, jl, ca, cl = it["jt"], it["jl"], it["ca"], it["cl"]
                qp0 = 32 * (h % 3)
                hq = h // 3
                ps_s = psS.tile([P, 512], F32, name="ps_s", tag="ps_s")
                nc.tensor.matmul(
                    ps_s[:jl, :cl],
                    k8s[qp0 : qp0 + 32, hq, :, jt * P : jt * P + jl],
                    q8s[qp0 : qp0 + 32, hq, :, ca : ca + cl],
                    start=True,
                    stop=True,
                    perf_mode=DR,
                )
                pt = phB.tile([P, 512], F16, name="pT", tag="pT", bufs=2 * LAG + 2)
                # exp split ACT (true exp) vs DVE (Schraudolph bit-trick), 5:3
                if exp_ctr % 8 < 5:
                    nc.scalar.activation(
                        pt[:jl, :cl],
                        ps_s[:jl, :cl],
                        mybir.ActivationFunctionType.Exp,
                        bias=0.0,
                        scale=ESC,
                    )
                else:
                    nc.vector.tensor_scalar(
                        pt[:jl, :cl].bitcast(I16),
                        ps_s[:jl, :cl],
                        EXP_A,
                        EXP_B,
                        mybir.AluOpType.mult,
                        mybir.AluOpType.add,
                    )
                exp_ctr += 1
                mk = it["mk"]
                if mk in ("T1", "T2"):
                    nc.gpsimd.tensor_tensor(
                        pt[:jl, 0:P],
                        pt[:jl, 0:P],
                        maskD[:jl, 0 if mk == "T1" else 1, :],
                        mybir.AluOpType.mult,
                    )
                elif mk == "TXT":
                    m0 = it["a"] - 512
                    nc.gpsimd.tensor_tensor(
                        pt[:jl, :cl],
                        pt[:jl, :cl],
                        maskTx[:jl, m0 : m0 + cl],
                        mybir.AluOpType.mult,
                    )
                return pt

            def issue_av(g, h, it, pt, ps_y, started):
                jt, jl = it["jt"], it["jl"]
                for ich, off, poff, pl in it["av"]:
                    if ich not in ps_y:
                        ps_y[ich] = psY.tile(
                            [P, 512], F32, name=f"ps_y{ich}", tag="ps_y"
                        )
                    nc.tensor.matmul(
                        ps_y[ich][:, off : off + pl],
                        v_ones[:jl, jt, h, :],
                        pt[:jl, poff : poff + pl],
                        start=ich not in started,
                        stop=it["last"],
                        skip_group_check=True,
                    )
                    started.add(ich)

            def issue_divides(g, h, ps_y):
                pof = D * (h % 2)
                ct = h // 2
                for ich, psy in ps_y.items():
                    i0, ilen = I_CHUNKS[ich]
                    rc = phB.tile([D, 512], F32, name="rc", tag="rc", bufs=4)
                    nc.vector.reciprocal_approx_fast(
                        out=rc[:, :ilen], in_=psy[0:D, :ilen]
                    )
                    nc.vector.tensor_tensor(
                        yT[pof : pof + D, ct, i0 : i0 + ilen],
                        psy[D : 2 * D, :ilen],
                        rc[:, :ilen],
                        mybir.AluOpType.mult,
                    )

            def outproj_groups(g):
                """Output-projection row tiles for group g."""
                return list(_GRP_ITS[g])

            def issue_outproj(itile):
                il = _jl(itile)
                ps_o = {
                    nch: psY.tile([P, 512], F32, name="ps_o", tag="ps_y")
                    for nch in range(2)
                }
                for kt in range(CT):
                    for nch in range(2):
                        nc.tensor.matmul(
                            ps_o[nch][:il, :384],
                            yT[:, kt, itile * P : itile * P + il],
                            wp_sb[:, kt, nch * 384 : (nch + 1) * 384],
                            start=(kt == 0),
                            stop=(kt == CT - 1),
                            skip_group_check=True,
                        )
                for nch in range(2):
                    o_sb = phC.tile([P, 384], F16, name="o_sb", tag="o_sb", bufs=4)
                    nc.scalar.copy(o_sb[:il, :], ps_o[nch][:il, :384])
                    nc.sync.dma_start(
                        out_d[itile * P : itile * P + il, nch * 384 : (nch + 1) * 384],
                        o_sb[:il, :],
                    )

            pending_proj = []  # outproj groups of the previous row-group
            for g in range(3):
                items = _grp_items(g)
                for h in range(NH):
                    pipeline = []  # (item, pt) awaiting their av matmul
                    ps_y = {}
                    started = set()
                    for idx, it in enumerate(items):
                        pt = issue_score(g, h, it)
                        pipeline.append((it, pt))
                        if len(pipeline) > LAG:
                            it2, pt2 = pipeline.pop(0)
                            issue_av(g, h, it2, pt2, ps_y, started)
                    for it2, pt2 in pipeline:
                        issue_av(g, h, it2, pt2, ps_y, started)
                    issue_divides(g, h, ps_y)
                    if h == 0:
                        while pending_proj:
                            issue_outproj(pending_proj.pop(0))
                pending_proj = outproj_groups(g)
            # tail: last group's output projection
            while pending_proj:
                issue_outproj(pending_proj.pop(0))

    nc.compile()
    return nc


def _build_mask_np(seg_starts, seg_ends):
    """True = masked. Mirrors reference._build_mask in numpy."""
    ML = 3 * T
    tril = np.tril(np.ones((T, T), dtype=bool))
    sl = np.tril(np.ones((T, T), dtype=bool), -1)
    m = np.zeros((L, L), dtype=bool)
    m[:ML, :ML] = True
    m[0:T, 0:T] = ~tril
    m[T : 2 * T, 0:T] = ~tril
    m[T : 2 * T, T : 2 * T] = ~sl
    m[T : 2 * T, 2 * T : 3 * T] = ~sl
    m[2 * T : 3 * T, 0:T] = ~tril
    m[2 * T : 3 * T, T : 2 * T] = ~tril
    m[2 * T : 3 * T, 2 * T : 3 * T] = ~sl
    m[:ML, ML:] = True
    frames = np.arange(T)[None, :, None]
    allowed = (frames >= seg_starts[:, None, :]) & (frames < seg_ends[:, None, :])
    mask = np.broadcast_to(m[None], (B, L, L)).copy()
    for row0, col_blocks in ((T, (0, 2, 3)), (2 * T, (1, 2, 3))):
        for j in col_blocks:
            c0 = ML + j * N
            mask[:, row0 : row0 + T, c0 : c0 + N] &= ~allowed
    return mask


def get_nc():
    global _NC
    if _NC is None:
        _NC = _build_program()
    return _NC


def make_in_maps(x, Wq, bq, Wk, bk, Wv, bv, Wp, bp, seg_starts, seg_ends):
    mask = _build_mask_np(np.asarray(seg_starts), np.asarray(seg_ends))
    r = np.arange(P)
    maskD = np.empty((P, 2, P), dtype=np.float16)
    maskD[:, 0, :] = (r[:, None] <= r[None, :]).astype(np.float16)  # tril.T
    maskD[:, 1, :] = (r[:, None] < r[None, :]).astype(np.float16)  # strict
    in_maps = []
    for core in range(8):
        b, g = core // 2, core % 2
        gs = slice(g * G, (g + 1) * G)
        allowT = ~mask[b].T  # [j, i]
        maskTx = np.ascontiguousarray(
            allowT[1536:1568, 512:1536].astype(np.float16)
        )
        f8 = ml_dtypes.float8_e4m3

        def split_c(a):  # [C, M] -> [P, 3, 2, M]
            M = a.shape[1]
            return np.ascontiguousarray(
                a.reshape(3, 2, P, M).transpose(2, 0, 1, 3)
            )

        in_maps.append(
            {
                "x8": split_c(x[b].T).astype(f8),
                "xT": np.ascontiguousarray(x[b].T).astype(np.float16),
                "w8q": split_c(Wq[gs, :].T * WSC).astype(f8),
                "w8k": split_c(Wk[gs, :].T * WSC).astype(f8),
                "wvT": np.ascontiguousarray(Wv[gs, :].T).astype(np.float16),
                "wpT": np.ascontiguousarray(Wp[:, gs].T).astype(np.float16),
                "bqP": np.ascontiguousarray((QSC * bq[gs]).reshape(CT, P).T),
                "bkP": np.ascontiguousarray((QSC * bk[gs]).reshape(CT, P).T),
                "bvB": np.broadcast_to(bv[gs], (P, G)).copy(),
                "maskD": maskD,
                "maskTxt": maskTx,
            }
        )
    return in_maps


def kernel(x, Wq, bq, Wk, bk, Wv, bv, Wp, bp, seg_starts, seg_ends, T_motion=None,
           N=None, _trace=False, **_unused):
    x = np.asarray(x, np.float32)
    args = [np.asarray(a, np.float32) for a in (Wq, bq, Wk, bk, Wv, bv, Wp, bp)]
    Wq, bq, Wk, bk, Wv, bv, Wp, bp = args
    nc = get_nc()
    in_maps = make_in_maps(x, Wq, bq, Wk, bk, Wv, bv, Wp, bp, seg_starts, seg_ends)
    res = run_bass_kernel_spmd(nc, in_maps, core_ids=list(range(8)), trace=_trace)
    parts = [np.asarray(r["out_part"], np.float32) for r in res.results]
    y = np.empty((B, L, C), np.float32)
    for b in range(B):
        y[b] = parts[2 * b] + parts[2 * b + 1] + bp
    if _trace:
        kernel.last_results = res
    return y
